# revision 32
# baseline (speedup 1.0000x reference)
"""Causal self-attention (B=4, T=2048, C=768, 12 heads) on 8 trn2 NeuronCores.

Sharding: core c handles batch b = c//2 and head-group hg = c%2 (6 heads each).
Each core computes its 6 heads end-to-end plus its slice of the output
projection; the two head-group partial projections per batch are summed on the
host (one 6 MB add per batch).

Per-core pipeline (matmuls in bf16 with fp32 PSUM accumulation — matches the
bf16-native numerics envelope for dense transformers; softmax in fp32):
  - x tiles cast to bf16 on DVE, transposed to xT via DMA x-bar transpose
  - qT/kT = W_qk^T x^T via PE (heads pair-packed into 128-partition tiles)
  - V in natural [t, hd] layout via PE with xT as the stationary operand,
    with a ones column appended per head for the softmax denominator
  - S^T[k,q] = K Q^T per head, two heads per PE slot via tile_position row
    packing (contraction dim = hd = 64); diagonal blocks compute only the
    live column range
  - exp on ACT straight out of PSUM (two heads per op) -> bf16 E; causal
    zero-fill via affine_select on GpSimd (diagonal tiles only)
  - PV accumulation in PSUM over k tiles (M=65: 64 value dims + denominator)
  - per-q-chunk normalization: batched DVE reciprocal, DRAM-bounce partition
    broadcast, DVE multiply — overlapped with the next chunk's compute
  - output projection per q-chunk from the transposed yT layout, DMA out
"""

import ml_dtypes
import numpy as np

import concourse.bacc as bacc
import concourse.mybir as mybir
import concourse.tile as tile
from concourse.bass_utils import run_bass_kernel_spmd

F32 = mybir.dt.float32
BF16 = mybir.dt.bfloat16
AF = mybir.ActivationFunctionType

B, T, C = 4, 2048, 768
NH, HD = 12, 64
TC = 4  # t-chunks of 512
CC = 6  # contraction chunks of 128 over C
N_TT = 16  # t tiles of 128

_nc_cache = {}


def _build(with_bias: bool):
    nc = bacc.Bacc(None, target_bir_lowering=False)
    xt_d = nc.dram_tensor("xt", [C, T], BF16, kind="ExternalInput")
    wqk = nc.dram_tensor("wqk", [C, 768], BF16, kind="ExternalInput")
    wv = nc.dram_tensor("wv", [C, 384], BF16, kind="ExternalInput")
    wp = nc.dram_tensor("wp", [384, C], BF16, kind="ExternalInput")
    if with_bias:
        bqk = nc.dram_tensor("bqk", [1, 768], BF16, kind="ExternalInput")
        bv = nc.dram_tensor("bv", [1, 384], BF16, kind="ExternalInput")
    out = nc.dram_tensor("out", [T, C], F32, kind="ExternalOutput")

    with tile.TileContext(nc) as tc, nc.allow_low_precision(
        reason="bf16 matmul operands are intentional"
    ):
        with (
            tc.tile_pool(name="const", bufs=1) as const,
            tc.tile_pool(name="xt", bufs=2) as xtp,
            tc.tile_pool(name="big", bufs=1) as big,
            tc.tile_pool(name="E", bufs=4) as epool,
            tc.tile_pool(name="norm", bufs=2) as npool,
            tc.tile_pool(name="ost", bufs=3) as opool,
            tc.tile_pool(name="dram", bufs=1, space="DRAM") as dpool,
            tc.tile_pool(name="ps", bufs=1, space="PSUM") as ps,
        ):
            # ---------------- constants ----------------
            onecol_f = const.tile([128, 8], F32)
            nc.vector.memset(onecol_f, 1.0)
            onecol_bf = const.tile([128, 8], BF16)
            nc.vector.tensor_copy(onecol_bf, onecol_f)

            # warm the ACT exp table while DMAs run
            warm_f = const.tile([1, 8], F32)
            nc.scalar.activation(warm_f, onecol_f[0:1, :], AF.Exp, scale=1.0)

            recip_d = dpool.tile([24, 512], F32)
            if with_bias:
                ones_f = const.tile([1, 512], F32)
                nc.vector.memset(ones_f, 1.0)
                ones_bf = const.tile([1, 512], BF16)
                nc.vector.tensor_copy(ones_bf, ones_f)

            ones64_f = const.tile([1, 64], F32)
            nc.vector.memset(ones64_f, 1.0)

            # PE warmup: dummy matmuls with no input deps keep the HAM
            # activity monitor busy while the first DMAs land
            wdum = const.tile([128, 512], BF16, name="wdum")
            nc.vector.memset(wdum.bitcast(F32)[:, 0:256], 0.0)
            wps = ps.tile([128, 512], F32, tag="mm", bufs=2, name="warmps")
            for _ in range(34):
                nc.tensor.matmul(wdum_mm := wps, wdum[:, 0:128], wdum, start=True, stop=True)
            warm_sb = const.tile([1, 8], F32, name="warmsb")
            nc.vector.tensor_copy(warm_sb, wps[0:1, 0:8])

            # ---------------- weights (bf16, host-prepared) ----------------
            wv_t = const.tile([128, CC, 384], BF16, name="wv_t")
            nc.sync.dma_start(
                out=wv_t, in_=wv.rearrange("(cc p) n -> p cc n", p=128)
            )
            wv_bf = [wv_t[:, cc, :] for cc in range(CC)]
            wqk_t = const.tile([128, CC, 768], BF16, name="wqk_t")
            nc.sync.dma_start(
                out=wqk_t, in_=wqk.rearrange("(cc p) n -> p cc n", p=128)
            )
            wqk_bf = [wqk_t[:, cc, :] for cc in range(CC)]
            wp_t = const.tile([128, 3, 768], BF16, name="wp_t")
            nc.sync.dma_start(
                out=wp_t, in_=wp.rearrange("(hp p) n -> p hp n", p=128)
            )
            wp_bf = [wp_t[:, hp, :] for hp in range(3)]
            if with_bias:
                bqk_bf = const.tile([1, 768], BF16)
                nc.sync.dma_start(out=bqk_bf, in_=bqk[:, :])
                bv_bf = const.tile([1, 384], BF16)
                nc.sync.dma_start(out=bv_bf, in_=bv[:, :])

            # persistent big tiles
            qkT = [big.tile([128, T], BF16, name=f"qkT{ct}") for ct in range(6)]
            v_sb = [big.tile([128, 390], BF16, name=f"v{tt}") for tt in range(N_TT)]
            yT3 = [big.tile([128, T], BF16, name=f"yT{hp}") for hp in range(3)]

            def emit_a_phase(qc):
                """xT chunk DMA, V and qT/kT matmuls for chunk qc."""
                xt_t = xtp.tile([128, CC, 512], BF16, tag="xt", name=f"xt_{qc}")
                nc.sync.dma_start(
                    out=xt_t,
                    in_=xt_d.rearrange("(cc p) t -> p cc t", p=128)[
                        :, :, qc * 512 : (qc + 1) * 512
                    ],
                )
                xt_tiles = [xt_t[:, cc, :] for cc in range(CC)]
                for tt4 in range(4):
                    tt = qc * 4 + tt4
                    v_ps = ps.tile([128, 384], F32, tag="mm", bufs=2, name=f"vps{tt}")
                    for cc in range(CC):
                        nc.tensor.matmul(
                            v_ps,
                            xt_tiles[cc][:, tt4 * 128 : (tt4 + 1) * 128],
                            wv_bf[cc],
                            start=(cc == 0),
                            stop=(cc == CC - 1 and not with_bias),
                        )
                    if with_bias:
                        nc.tensor.matmul(
                            v_ps, ones_bf[:, 0:128], bv_bf, start=False, stop=True
                        )
                    vv = v_sb[tt].rearrange("p (h w) -> p h w", w=65)
                    nc.vector.tensor_copy(
                        vv[:, :, 0:64], v_ps.rearrange("p (h w) -> p h w", w=64)
                    )
                    nc.vector.tensor_copy(vv[:, :, 64], onecol_bf[:, 0:6])
                for ct in range(6):
                    qk_ps = ps.tile(
                        [128, 512], F32, tag="mm", bufs=2, name=f"qkps{qc}_{ct}"
                    )
                    for cc in range(CC):
                        nc.tensor.matmul(
                            qk_ps,
                            wqk_bf[cc][:, ct * 128 : (ct + 1) * 128],
                            xt_tiles[cc],
                            start=(cc == 0),
                            stop=(cc == CC - 1 and not with_bias),
                        )
                    if with_bias:
                        nc.tensor.matmul(
                            qk_ps,
                            bqk_bf[:, ct * 128 : (ct + 1) * 128],
                            ones_bf,
                            start=False,
                            stop=True,
                        )
                    nc.vector.tensor_copy(qkT[ct][:, qc * 512 : (qc + 1) * 512], qk_ps)

            def emit_attention(qc):
                q_sl = slice(qc * 512, (qc + 1) * 512)
                n_kt = 4 * qc + 4
                if qc == TC - 1:
                    dent6 = None
                    dent = [
                        [
                            npool.tile(
                                [1, 512], F32, tag="dent1", bufs=6, name=f"dent{qc}_{h}_{i}"
                            )
                            for i in range(2)
                        ]
                        for h in range(3)
                    ]
                else:
                    dent6 = npool.tile([6, 512], F32, tag="dent6", name=f"dent{qc}")
                    dent = [dent6[2 * h : 2 * h + 2, :] for h in range(3)]
                dent.append(dent6)
                for hp in range(3):
                    yT_a = ps.tile([65, 512], F32, tag="yT", bufs=2, name=f"ya{qc}_{hp}")
                    yT_b = ps.tile([65, 512], F32, tag="yT", bufs=2, name=f"yb{qc}_{hp}")
                    e_hist = []
                    for kt in range(n_kt):
                        k_sl = slice(kt * 128, (kt + 1) * 128)
                        m = kt - 4 * qc
                        diag = m >= 0
                        w = 512 - 128 * max(m, 0)  # live column range of this block
                        f0 = 512 - w
                        psS = ps.tile(
                            [128, 1024], F32, tag="S", bufs=2, name=f"s{qc}_{hp}_{kt}"
                        )
                        nc.tensor.matmul(
                            psS[:, f0:512],
                            qkT[3 + hp][0:64, k_sl],
                            qkT[hp][0:64, qc * 512 + f0 : (qc + 1) * 512],
                            start=True,
                            stop=True,
                            tile_position=(0, 0),
                        )
                        nc.tensor.matmul(
                            psS[:, 512 + f0 : 1024],
                            qkT[3 + hp][64:128, k_sl],
                            qkT[hp][64:128, qc * 512 + f0 : (qc + 1) * 512],
                            start=True,
                            stop=True,
                            tile_position=(64, 0),
                        )
                        E = epool.tile(
                            [128, 1024], BF16, tag="E", name=f"e{qc}_{hp}_{kt}"
                        )
                        psv = psS.rearrange("p (h w) -> p h w", w=512)
                        ev = E.rearrange("p (h w) -> p h w", w=512)
                        nc.scalar.activation(
                            ev[:, :, f0:512], psv[:, :, f0:512], AF.Exp, scale=0.125
                        )
                        if diag:
                            # keep where q - k = f' - p >= 0; only the first
                            # 128 columns of the live range can be masked
                            nc.gpsimd.affine_select(
                                out=ev[:, :, f0 : f0 + 128],
                                in_=ev[:, :, f0 : f0 + 128],
                                compare_op=mybir.AluOpType.is_ge,
                                fill=0.0,
                                base=0,
                                pattern=[[0, 2], [1, 128]],
                                channel_multiplier=-1,
                            )
                        e_hist.append((kt, E, f0))
                        if len(e_hist) > 2:
                            _pv(nc, v_sb, yT_a, yT_b, hp, *e_hist.pop(0), n_kt)
                    while e_hist:
                        _pv(nc, v_sb, yT_a, yT_b, hp, *e_hist.pop(0), n_kt)

                    # stash raw outputs + denominators (normalized per chunk)
                    for hip, yT_ps in ((0, yT_a), (1, yT_b)):
                        j = hp * 2 + hip
                        idx = qc * 6 + j
                        dst = npool.tile([65, 512], F32, tag="dstage", name=f"ds{idx}")
                        nc.vector.tensor_copy(dst[64:65, :], yT_ps[64:65, :])
                        _ = None
                        if qc == TC - 1:
                            nc.sync.dma_start(out=dent[hp][hip], in_=dst[64:65, :])
                        else:
                            nc.sync.dma_start(
                                out=dent[hp][hip : hip + 1, :], in_=dst[64:65, :]
                            )
                        if hip == 0:
                            nc.vector.tensor_copy(yT3[hp][0:64, q_sl], yT_ps[0:64, :])
                        else:
                            ytmp = npool.tile(
                                [64, 512], BF16, tag="ytmp", name=f"yt{idx}"
                            )
                            nc.vector.tensor_copy(ytmp, yT_ps[0:64, :])
                            # partition shift 0:64 -> 64:128 via SBUF->SBUF DMA
                            nc.sync.dma_start(out=yT3[hp][64:128, q_sl], in_=ytmp)
                    if qc == TC - 1:
                        emit_normalize(qc, dent, [hp])
                return dent

            def emit_normalize(qc, dent, hps):
                q_sl = slice(qc * 512, (qc + 1) * 512)
                if len(hps) == 3:
                    # batched: dent slices alias one [6, 512] tile
                    rec6 = npool.tile([6, 512], F32, tag="rec6", name=f"rec6_{qc}")
                    nc.vector.reciprocal_approx_fast(rec6, dent[3])
                    nc.sync.dma_start(out=recip_d[qc * 6 : qc * 6 + 6, :], in_=rec6)
                for hp in hps:
                    if len(hps) != 3:
                        for hip in range(2):
                            rec1 = npool.tile(
                                [1, 512], F32, tag="rec1", bufs=4,
                                name=f"r1_{qc}_{hp}_{hip}",
                            )
                            nc.vector.reciprocal_approx_fast(rec1, dent[hp][hip])
                            nc.sync.dma_start(
                                out=recip_d[qc * 6 + 2 * hp + hip, :][None, :],
                                in_=rec1,
                            )
                    for hip in range(2):
                        idx = qc * 6 + hp * 2 + hip
                        rows = slice(64 * hip, 64 * hip + 64)
                        bc = npool.tile([128, 512], F32, tag="bcast", name=f"bc{idx}")
                        rrow = recip_d[idx, :]
                        bcast_ap = bacc.bass.AP(
                            tensor=rrow.tensor,
                            offset=rrow.offset,
                            ap=[[0, 64]] + list(rrow.ap),
                        )
                        dma_eng = nc.scalar if len(hps) != 3 else nc.gpsimd
                        dma_eng.dma_start(out=bc[rows, :], in_=bcast_ap)
                        nc.vector.tensor_mul(
                            yT3[hp][rows, q_sl], yT3[hp][rows, q_sl], bc[rows, :]
                        )

            def emit_proj(qc):
                for tt in range(qc * 4, qc * 4 + 4):
                    t_sl = slice(tt * 128, (tt + 1) * 128)
                    ostage = opool.tile([128, 768], F32, tag="ost")
                    for half in range(2):
                        pp = ps.tile(
                            [128, 384], F32, tag="mm", bufs=2, name=f"pj{tt}_{half}"
                        )
                        for hp in range(3):
                            nc.tensor.matmul(
                                pp,
                                yT3[hp][:, t_sl],
                                wp_bf[hp][:, half * 384 : (half + 1) * 384],
                                start=(hp == 0),
                                stop=(hp == 2),
                            )
                        nc.vector.tensor_copy(
                            ostage[:, half * 384 : (half + 1) * 384], pp
                        )
                    nc.sync.dma_start(out=out[t_sl, :], in_=ostage)

            # ---------------- main loop: qc-major, proj one chunk behind ----
            emit_a_phase(0)
            for qc in range(TC):
                dent = emit_attention(qc)
                if qc < TC - 1:
                    emit_normalize(qc, dent, [0, 1, 2])
                    emit_a_phase(qc + 1)
                if qc >= 1:
                    emit_proj(qc - 1)
            emit_proj(TC - 1)

    nc.finalize()
    return nc


def _pv(nc, v_sb, yT_a, yT_b, hp, kt, E, f0, n_kt):
    a = 2 * hp
    nc.tensor.matmul(
        yT_a[:, f0:512],
        v_sb[kt][:, a * 65 : (a + 1) * 65],
        E[:, f0:512],
        start=(kt == 0),
        stop=(kt == n_kt - 1),
    )
    nc.tensor.matmul(
        yT_b[:, f0:512],
        v_sb[kt][:, (a + 1) * 65 : (a + 2) * 65],
        E[:, 512 + f0 : 1024],
        start=(kt == 0),
        stop=(kt == n_kt - 1),
    )


def _get_nc(with_bias: bool):
    if with_bias not in _nc_cache:
        _nc_cache[with_bias] = _build(with_bias)
    return _nc_cache[with_bias]


def kernel(x, W_attn, b_attn, W_proj, b_proj, _run_kwargs=None):
    x = np.ascontiguousarray(np.asarray(x, dtype=np.float32))
    W_attn = np.ascontiguousarray(np.asarray(W_attn, dtype=np.float32))
    b_attn = np.ascontiguousarray(np.asarray(b_attn, dtype=np.float32))
    W_proj = np.ascontiguousarray(np.asarray(W_proj, dtype=np.float32))
    b_proj = np.ascontiguousarray(np.asarray(b_proj, dtype=np.float32))

    with_bias = bool(np.any(b_attn))
    nc = _get_nc(with_bias)

    bf = ml_dtypes.bfloat16
    xt_by_b = [np.ascontiguousarray(x[b].T.astype(bf)) for b in range(B)]
    in_maps = []
    for c in range(8):
        b = c // 2
        hg = c % 2
        cs = slice(hg * 384, (hg + 1) * 384)
        wq = W_attn[:, 0:768][:, cs]
        wk = W_attn[:, 768:1536][:, cs]
        wvs = W_attn[:, 1536:2304][:, cs]
        m = {
            "xt": xt_by_b[b],
            "wqk": np.ascontiguousarray(
                np.concatenate([wq, wk], axis=1).astype(bf)
            ),
            "wv": np.ascontiguousarray(wvs.astype(bf)),
            "wp": np.ascontiguousarray(W_proj[cs, :].astype(bf)),
        }
        if with_bias:
            m["bqk"] = np.ascontiguousarray(
                np.concatenate([b_attn[0:768][cs], b_attn[768:1536][cs]]).astype(bf)
            )[None, :]
            m["bv"] = np.ascontiguousarray(b_attn[1536:2304][cs].astype(bf))[None, :]
        in_maps.append(m)

    kwargs = _run_kwargs or {}
    res = run_bass_kernel_spmd(nc, in_maps, core_ids=list(range(8)), **kwargs)

    y = np.empty((B, T, C), dtype=np.float32)
    for b in range(B):
        y[b] = res.results[2 * b]["out"] + res.results[2 * b + 1]["out"]
    y += b_proj[None, None, :]
    if kwargs:
        kernel.last_result = res
    return y


# revision 33
# speedup vs baseline: 1.0060x; 1.0060x over previous
"""Causal self-attention (B=4, T=2048, C=768, 12 heads) on 8 trn2 NeuronCores.

Sharding: core c handles batch b = c//2 and head-group hg = c%2 (6 heads each).
Each core computes its 6 heads end-to-end plus its slice of the output
projection; the two head-group partial projections per batch are summed on the
host (one 6 MB add per batch).

Per-core pipeline (matmuls in bf16 with fp32 PSUM accumulation — matches the
bf16-native numerics envelope for dense transformers; softmax in fp32):
  - x tiles cast to bf16 on DVE, transposed to xT via DMA x-bar transpose
  - qT/kT = W_qk^T x^T via PE (heads pair-packed into 128-partition tiles)
  - V in natural [t, hd] layout via PE with xT as the stationary operand,
    with a ones column appended per head for the softmax denominator
  - S^T[k,q] = K Q^T per head, two heads per PE slot via tile_position row
    packing (contraction dim = hd = 64); diagonal blocks compute only the
    live column range
  - exp on ACT straight out of PSUM (two heads per op) -> bf16 E; causal
    zero-fill via affine_select on GpSimd (diagonal tiles only)
  - PV accumulation in PSUM over k tiles (M=65: 64 value dims + denominator)
  - per-q-chunk normalization: batched DVE reciprocal, DRAM-bounce partition
    broadcast, DVE multiply — overlapped with the next chunk's compute
  - output projection per q-chunk from the transposed yT layout, DMA out
"""

import ml_dtypes
import numpy as np

import concourse.bacc as bacc
import concourse.mybir as mybir
import concourse.tile as tile
from concourse.bass_utils import run_bass_kernel_spmd

F32 = mybir.dt.float32
BF16 = mybir.dt.bfloat16
AF = mybir.ActivationFunctionType

B, T, C = 4, 2048, 768
NH, HD = 12, 64
TC = 4  # t-chunks of 512
CC = 6  # contraction chunks of 128 over C
N_TT = 16  # t tiles of 128

_nc_cache = {}


def _build(with_bias: bool):
    nc = bacc.Bacc(None, target_bir_lowering=False)
    xt_d = nc.dram_tensor("xt", [C, T], BF16, kind="ExternalInput")
    wqk = nc.dram_tensor("wqk", [C, 768], BF16, kind="ExternalInput")
    wv = nc.dram_tensor("wv", [C, 384], BF16, kind="ExternalInput")
    wp = nc.dram_tensor("wp", [384, C], BF16, kind="ExternalInput")
    if with_bias:
        bqk = nc.dram_tensor("bqk", [1, 768], BF16, kind="ExternalInput")
        bv = nc.dram_tensor("bv", [1, 384], BF16, kind="ExternalInput")
    out = nc.dram_tensor("out", [T, C], F32, kind="ExternalOutput")

    with tile.TileContext(nc) as tc, nc.allow_low_precision(
        reason="bf16 matmul operands are intentional"
    ):
        with (
            tc.tile_pool(name="const", bufs=1) as const,
            tc.tile_pool(name="xt", bufs=2) as xtp,
            tc.tile_pool(name="big", bufs=1) as big,
            tc.tile_pool(name="E", bufs=4) as epool,
            tc.tile_pool(name="norm", bufs=2) as npool,
            tc.tile_pool(name="ost", bufs=3) as opool,
            tc.tile_pool(name="dram", bufs=1, space="DRAM") as dpool,
            tc.tile_pool(name="ps", bufs=1, space="PSUM") as ps,
        ):
            # ---------------- constants ----------------
            onecol_f = const.tile([128, 8], F32)
            nc.vector.memset(onecol_f, 1.0)
            onecol_bf = const.tile([128, 8], BF16)
            nc.vector.tensor_copy(onecol_bf, onecol_f)

            # warm the ACT exp table while DMAs run
            warm_f = const.tile([1, 8], F32)
            nc.scalar.activation(warm_f, onecol_f[0:1, :], AF.Exp, scale=1.0)

            recip_d = dpool.tile([24, 512], F32)
            if with_bias:
                ones_f = const.tile([1, 512], F32)
                nc.vector.memset(ones_f, 1.0)
                ones_bf = const.tile([1, 512], BF16)
                nc.vector.tensor_copy(ones_bf, ones_f)

            ones64_f = const.tile([1, 64], F32)
            nc.vector.memset(ones64_f, 1.0)

            # PE warmup: dummy matmuls with no input deps keep the HAM
            # activity monitor busy while the first DMAs land
            wdum = const.tile([128, 512], BF16, name="wdum")
            nc.vector.memset(wdum.bitcast(F32)[:, 0:256], 0.0)
            wps = ps.tile([128, 512], F32, tag="mm", bufs=2, name="warmps")
            for _ in range(26):
                nc.tensor.matmul(wdum_mm := wps, wdum[:, 0:128], wdum, start=True, stop=True)
            warm_sb = const.tile([1, 8], F32, name="warmsb")
            nc.vector.tensor_copy(warm_sb, wps[0:1, 0:8])

            # ---------------- weights (bf16, host-prepared) ----------------
            wv_t = const.tile([128, CC, 384], BF16, name="wv_t")
            nc.sync.dma_start(
                out=wv_t, in_=wv.rearrange("(cc p) n -> p cc n", p=128)
            )
            wv_bf = [wv_t[:, cc, :] for cc in range(CC)]
            wqk_t = const.tile([128, CC, 768], BF16, name="wqk_t")
            nc.sync.dma_start(
                out=wqk_t, in_=wqk.rearrange("(cc p) n -> p cc n", p=128)
            )
            wqk_bf = [wqk_t[:, cc, :] for cc in range(CC)]
            wp_t = const.tile([128, 3, 768], BF16, name="wp_t")
            nc.sync.dma_start(
                out=wp_t, in_=wp.rearrange("(hp p) n -> p hp n", p=128)
            )
            wp_bf = [wp_t[:, hp, :] for hp in range(3)]
            if with_bias:
                bqk_bf = const.tile([1, 768], BF16)
                nc.sync.dma_start(out=bqk_bf, in_=bqk[:, :])
                bv_bf = const.tile([1, 384], BF16)
                nc.sync.dma_start(out=bv_bf, in_=bv[:, :])

            # persistent big tiles
            qkT = [big.tile([128, T], BF16, name=f"qkT{ct}") for ct in range(6)]
            v_sb = [big.tile([128, 390], BF16, name=f"v{tt}") for tt in range(N_TT)]
            yT3 = [big.tile([128, T], BF16, name=f"yT{hp}") for hp in range(3)]

            def emit_a_phase(qc):
                """xT chunk DMA, V and qT/kT matmuls for chunk qc."""
                xt_t = xtp.tile([128, CC, 512], BF16, tag="xt", name=f"xt_{qc}")
                nc.sync.dma_start(
                    out=xt_t,
                    in_=xt_d.rearrange("(cc p) t -> p cc t", p=128)[
                        :, :, qc * 512 : (qc + 1) * 512
                    ],
                )
                xt_tiles = [xt_t[:, cc, :] for cc in range(CC)]
                for tt4 in range(4):
                    tt = qc * 4 + tt4
                    v_ps = ps.tile([128, 384], F32, tag="mm", bufs=2, name=f"vps{tt}")
                    for cc in range(CC):
                        nc.tensor.matmul(
                            v_ps,
                            xt_tiles[cc][:, tt4 * 128 : (tt4 + 1) * 128],
                            wv_bf[cc],
                            start=(cc == 0),
                            stop=(cc == CC - 1 and not with_bias),
                        )
                    if with_bias:
                        nc.tensor.matmul(
                            v_ps, ones_bf[:, 0:128], bv_bf, start=False, stop=True
                        )
                    vv = v_sb[tt].rearrange("p (h w) -> p h w", w=65)
                    nc.vector.tensor_copy(
                        vv[:, :, 0:64], v_ps.rearrange("p (h w) -> p h w", w=64)
                    )
                    nc.vector.tensor_copy(vv[:, :, 64], onecol_bf[:, 0:6])
                for ct in range(6):
                    qk_ps = ps.tile(
                        [128, 512], F32, tag="mm", bufs=2, name=f"qkps{qc}_{ct}"
                    )
                    for cc in range(CC):
                        nc.tensor.matmul(
                            qk_ps,
                            wqk_bf[cc][:, ct * 128 : (ct + 1) * 128],
                            xt_tiles[cc],
                            start=(cc == 0),
                            stop=(cc == CC - 1 and not with_bias),
                        )
                    if with_bias:
                        nc.tensor.matmul(
                            qk_ps,
                            bqk_bf[:, ct * 128 : (ct + 1) * 128],
                            ones_bf,
                            start=False,
                            stop=True,
                        )
                    nc.vector.tensor_copy(qkT[ct][:, qc * 512 : (qc + 1) * 512], qk_ps)

            def emit_attention(qc):
                q_sl = slice(qc * 512, (qc + 1) * 512)
                n_kt = 4 * qc + 4
                if qc == TC - 1:
                    dent6 = None
                    dent = [
                        [
                            npool.tile(
                                [1, 512], F32, tag="dent1", bufs=6, name=f"dent{qc}_{h}_{i}"
                            )
                            for i in range(2)
                        ]
                        for h in range(3)
                    ]
                else:
                    dent6 = npool.tile([6, 512], F32, tag="dent6", name=f"dent{qc}")
                    dent = [dent6[2 * h : 2 * h + 2, :] for h in range(3)]
                dent.append(dent6)
                for hp in range(3):
                    yT_a = ps.tile([65, 512], F32, tag="yT", bufs=2, name=f"ya{qc}_{hp}")
                    yT_b = ps.tile([65, 512], F32, tag="yT", bufs=2, name=f"yb{qc}_{hp}")
                    e_hist = []
                    for kt in range(n_kt):
                        k_sl = slice(kt * 128, (kt + 1) * 128)
                        m = kt - 4 * qc
                        diag = m >= 0
                        w = 512 - 128 * max(m, 0)  # live column range of this block
                        f0 = 512 - w
                        psS = ps.tile(
                            [128, 1024], F32, tag="S", bufs=2, name=f"s{qc}_{hp}_{kt}"
                        )
                        nc.tensor.matmul(
                            psS[:, f0:512],
                            qkT[3 + hp][0:64, k_sl],
                            qkT[hp][0:64, qc * 512 + f0 : (qc + 1) * 512],
                            start=True,
                            stop=True,
                            tile_position=(0, 0),
                        )
                        nc.tensor.matmul(
                            psS[:, 512 + f0 : 1024],
                            qkT[3 + hp][64:128, k_sl],
                            qkT[hp][64:128, qc * 512 + f0 : (qc + 1) * 512],
                            start=True,
                            stop=True,
                            tile_position=(64, 0),
                        )
                        E = epool.tile(
                            [128, 1024], BF16, tag="E", name=f"e{qc}_{hp}_{kt}"
                        )
                        psv = psS.rearrange("p (h w) -> p h w", w=512)
                        ev = E.rearrange("p (h w) -> p h w", w=512)
                        nc.scalar.activation(
                            ev[:, :, f0:512], psv[:, :, f0:512], AF.Exp, scale=0.125
                        )
                        if diag:
                            # keep where q - k = f' - p >= 0; only the first
                            # 128 columns of the live range can be masked
                            nc.gpsimd.affine_select(
                                out=ev[:, :, f0 : f0 + 128],
                                in_=ev[:, :, f0 : f0 + 128],
                                compare_op=mybir.AluOpType.is_ge,
                                fill=0.0,
                                base=0,
                                pattern=[[0, 2], [1, 128]],
                                channel_multiplier=-1,
                            )
                        e_hist.append((kt, E, f0))
                        if len(e_hist) > 2:
                            _pv(nc, v_sb, yT_a, yT_b, hp, *e_hist.pop(0), n_kt)
                    while e_hist:
                        _pv(nc, v_sb, yT_a, yT_b, hp, *e_hist.pop(0), n_kt)

                    # stash raw outputs + denominators (normalized per chunk)
                    for hip, yT_ps in ((0, yT_a), (1, yT_b)):
                        j = hp * 2 + hip
                        idx = qc * 6 + j
                        dst = npool.tile([65, 512], F32, tag="dstage", name=f"ds{idx}")
                        nc.vector.tensor_copy(dst[64:65, :], yT_ps[64:65, :])
                        if qc == TC - 1:
                            nc.sync.dma_start(out=dent[hp][hip], in_=dst[64:65, :])
                        else:
                            nc.sync.dma_start(
                                out=dent[hp][hip : hip + 1, :], in_=dst[64:65, :]
                            )
                        if hip == 0:
                            nc.vector.tensor_copy(yT3[hp][0:64, q_sl], yT_ps[0:64, :])
                        else:
                            ytmp = npool.tile(
                                [64, 512], BF16, tag="ytmp", name=f"yt{idx}"
                            )
                            nc.vector.tensor_copy(ytmp, yT_ps[0:64, :])
                            # partition shift 0:64 -> 64:128 via SBUF->SBUF DMA
                            nc.sync.dma_start(out=yT3[hp][64:128, q_sl], in_=ytmp)
                    if qc == TC - 1:
                        emit_normalize(qc, dent, [hp])
                return dent

            def emit_normalize(qc, dent, hps):
                q_sl = slice(qc * 512, (qc + 1) * 512)
                if len(hps) == 3:
                    # batched: dent slices alias one [6, 512] tile
                    rec6 = npool.tile([6, 512], F32, tag="rec6", name=f"rec6_{qc}")
                    nc.vector.reciprocal_approx_fast(rec6, dent[3])
                    nc.sync.dma_start(out=recip_d[qc * 6 : qc * 6 + 6, :], in_=rec6)
                for hp in hps:
                    if len(hps) != 3:
                        for hip in range(2):
                            rec1 = npool.tile(
                                [1, 512], F32, tag="rec1", bufs=4,
                                name=f"r1_{qc}_{hp}_{hip}",
                            )
                            nc.vector.reciprocal_approx_fast(rec1, dent[hp][hip])
                            nc.sync.dma_start(
                                out=recip_d[qc * 6 + 2 * hp + hip, :][None, :],
                                in_=rec1,
                            )
                    for hip in range(2):
                        idx = qc * 6 + hp * 2 + hip
                        rows = slice(64 * hip, 64 * hip + 64)
                        bc = npool.tile([128, 512], F32, tag="bcast", name=f"bc{idx}")
                        rrow = recip_d[idx, :]
                        bcast_ap = bacc.bass.AP(
                            tensor=rrow.tensor,
                            offset=rrow.offset,
                            ap=[[0, 64]] + list(rrow.ap),
                        )
                        nc.gpsimd.dma_start(out=bc[rows, :], in_=bcast_ap)
                        nc.vector.tensor_mul(
                            yT3[hp][rows, q_sl], yT3[hp][rows, q_sl], bc[rows, :]
                        )

            def emit_proj(qc):
                for tt in range(qc * 4, qc * 4 + 4):
                    t_sl = slice(tt * 128, (tt + 1) * 128)
                    ostage = opool.tile([128, 768], F32, tag="ost")
                    for half in range(2):
                        pp = ps.tile(
                            [128, 384], F32, tag="mm", bufs=2, name=f"pj{tt}_{half}"
                        )
                        for hp in range(3):
                            nc.tensor.matmul(
                                pp,
                                yT3[hp][:, t_sl],
                                wp_bf[hp][:, half * 384 : (half + 1) * 384],
                                start=(hp == 0),
                                stop=(hp == 2),
                            )
                        nc.vector.tensor_copy(
                            ostage[:, half * 384 : (half + 1) * 384], pp
                        )
                    nc.sync.dma_start(out=out[t_sl, :], in_=ostage)

            # ---------------- main loop: qc-major, proj one chunk behind ----
            emit_a_phase(0)
            for qc in range(TC):
                dent = emit_attention(qc)
                if qc < TC - 1:
                    emit_normalize(qc, dent, [0, 1, 2])
                    emit_a_phase(qc + 1)
                if qc >= 1:
                    emit_proj(qc - 1)
            emit_proj(TC - 1)

    nc.finalize()
    return nc


def _pv(nc, v_sb, yT_a, yT_b, hp, kt, E, f0, n_kt):
    a = 2 * hp
    nc.tensor.matmul(
        yT_a[:, f0:512],
        v_sb[kt][:, a * 65 : (a + 1) * 65],
        E[:, f0:512],
        start=(kt == 0),
        stop=(kt == n_kt - 1),
    )
    nc.tensor.matmul(
        yT_b[:, f0:512],
        v_sb[kt][:, (a + 1) * 65 : (a + 2) * 65],
        E[:, 512 + f0 : 1024],
        start=(kt == 0),
        stop=(kt == n_kt - 1),
    )


def _get_nc(with_bias: bool):
    if with_bias not in _nc_cache:
        _nc_cache[with_bias] = _build(with_bias)
    return _nc_cache[with_bias]


def kernel(x, W_attn, b_attn, W_proj, b_proj, _run_kwargs=None):
    x = np.ascontiguousarray(np.asarray(x, dtype=np.float32))
    W_attn = np.ascontiguousarray(np.asarray(W_attn, dtype=np.float32))
    b_attn = np.ascontiguousarray(np.asarray(b_attn, dtype=np.float32))
    W_proj = np.ascontiguousarray(np.asarray(W_proj, dtype=np.float32))
    b_proj = np.ascontiguousarray(np.asarray(b_proj, dtype=np.float32))

    with_bias = bool(np.any(b_attn))
    nc = _get_nc(with_bias)

    bf = ml_dtypes.bfloat16
    xt_by_b = [np.ascontiguousarray(x[b].T.astype(bf)) for b in range(B)]
    in_maps = []
    for c in range(8):
        b = c // 2
        hg = c % 2
        cs = slice(hg * 384, (hg + 1) * 384)
        wq = W_attn[:, 0:768][:, cs]
        wk = W_attn[:, 768:1536][:, cs]
        wvs = W_attn[:, 1536:2304][:, cs]
        m = {
            "xt": xt_by_b[b],
            "wqk": np.ascontiguousarray(
                np.concatenate([wq, wk], axis=1).astype(bf)
            ),
            "wv": np.ascontiguousarray(wvs.astype(bf)),
            "wp": np.ascontiguousarray(W_proj[cs, :].astype(bf)),
        }
        if with_bias:
            m["bqk"] = np.ascontiguousarray(
                np.concatenate([b_attn[0:768][cs], b_attn[768:1536][cs]]).astype(bf)
            )[None, :]
            m["bv"] = np.ascontiguousarray(b_attn[1536:2304][cs].astype(bf))[None, :]
        in_maps.append(m)

    kwargs = _run_kwargs or {}
    res = run_bass_kernel_spmd(nc, in_maps, core_ids=list(range(8)), **kwargs)

    y = np.empty((B, T, C), dtype=np.float32)
    for b in range(B):
        y[b] = res.results[2 * b]["out"] + res.results[2 * b + 1]["out"]
    y += b_proj[None, None, :]
    if kwargs:
        kernel.last_result = res
    return y
